# revision 3
# baseline (speedup 1.0000x reference)
"""Multi-head causal attention (B=2, T=2048, H=16, D=64, C=1024) on 8 trn2 cores.

Sharding: tensor-parallel over heads. Each core owns 2 heads (both batches):
  - computes Q^T/K^T/V^T for its heads over all 4096 tokens
  - causal attention in transposed orientation (S^T[k,q]) so no P transpose
  - partial output projection outT_partial[c, t] = Wo_slice^T @ O^T
Host sums the 8 partials (the "all-reduce"), adds bias, transposes back.

v2 layout/scheduling:
  - QKV projection work interleaved into the attention stream so the PE has
    independent matmuls to run while ACT computes exp (keeps HAM warm, kills
    dependency stalls)
  - exp over ktile PAIRS (one ACT instr per pair, both heads) to amortize the
    per-instruction overhead; causal triangle + stale-region masking fused
    into one gpsimd affine_select per diagonal pair
  - rowsum reciprocal merged across both heads (one 128-partition recip)
  - 8KB/partition contiguous DMA layouts (xt[p][tb][ct][t], out[p][tb][ct][t])
  - bf16 partial outputs (halves write traffic; host accumulates in fp32)
"""

import sys

sys.path.insert(0, "/opt/trn_rl_repo")

import ml_dtypes
import numpy as np

import concourse.bacc as bacc
import concourse.mybir as mybir
import concourse.tile as tile
from concourse.bass_utils import run_bass_kernel_spmd

B, T, C = 2, 2048, 1024
H, D = 16, 64
NT = B * T  # 4096 flattened tokens
N_CORES = 8
HPC = H // N_CORES  # 2 heads per core
FPC = HPC * D  # 128 features per core
CT = C // 128  # 8 contraction tiles for projections
TBLK = 512  # token block
NTB = NT // TBLK  # 8 token blocks
QB = T // TBLK  # 4 query blocks per batch
KT = T // 128  # 16 key tiles per batch

F32 = mybir.dt.float32
BF16 = mybir.dt.bfloat16


def build_program():
    nc = bacc.Bacc("TRN2", target_bir_lowering=False, debug=False)

    xt_d = nc.declare_dram_parameter("xt", [128, NTB, CT, TBLK], BF16, isOutput=False)
    wq_d = nc.declare_dram_parameter("wq", [128, CT, FPC], BF16, isOutput=False)
    wk_d = nc.declare_dram_parameter("wk", [128, CT, FPC], BF16, isOutput=False)
    wv_d = nc.declare_dram_parameter("wv", [128, CT, FPC], BF16, isOutput=False)
    wo_d = nc.declare_dram_parameter("wo", [FPC, C], BF16, isOutput=False)
    out_d = nc.declare_dram_parameter("outT", [128, NTB, CT, TBLK], BF16, isOutput=True)

    with tile.TileContext(nc) as tc:
        with (
            tc.tile_pool(name="slabs", bufs=1) as slabs,
            tc.tile_pool(name="xtp", bufs=2) as xtp,
            tc.tile_pool(name="esp", bufs=2) as esp,
            tc.tile_pool(name="vtp", bufs=2) as vtp,
            tc.tile_pool(name="rinp", bufs=2) as rinp,
            tc.tile_pool(name="outp", bufs=2) as outp,
            tc.tile_pool(name="psS", bufs=1, space="PSUM") as psS,  # 4 banks
            tc.tile_pool(name="psO", bufs=1, space="PSUM") as psO,  # 2 banks
            tc.tile_pool(name="psA", bufs=2, space="PSUM") as psA,  # 2 banks
        ):
            # ---- persistent slabs
            qT = slabs.tile([128, NT], BF16, tag="qT")  # [2h*64d, t]
            kT = slabs.tile([128, NT], BF16, tag="kT")
            # V natural layout: per ktile_global: [128k, (ones | V_h0 | V_h1 | ones)]
            # PV stationary h0 = [:, ktg, 0:2, :] = [ones|V_h0] -> rowsum rows 0:64, O 64:128
            #               h1 = [:, ktg, 2:4, :] = [V_h1|ones] -> O rows 0:64, rowsum 64:128
            vN = slabs.tile([128, NTB * 4, 4, 64], BF16, tag="vN")
            oN = slabs.tile([128, NT], BF16, tag="oN")  # normalized O^T
            wq_s = slabs.tile([128, CT, FPC], BF16, tag="wq")
            wk_s = slabs.tile([128, CT, FPC], BF16, tag="wk")
            wv_s = slabs.tile([128, CT, FPC], BF16, tag="wv")
            wo_s = slabs.tile([128, C], BF16, tag="wo")  # [f, c]
            ident = slabs.tile([128, 128], BF16, tag="ident")

            # ---- constants
            from concourse.masks import make_identity
            make_identity(nc, ident[:])
            # ones columns of vN (constant for the whole run)
            nc.gpsimd.memset(vN[:, :, 0, :], 1.0)
            nc.gpsimd.memset(vN[:, :, 3, :], 1.0)

            # ---- weight loads
            nc.sync.dma_start(wq_s[:], wq_d[:])
            nc.sync.dma_start(wk_s[:], wk_d[:])
            nc.sync.dma_start(wv_s[:], wv_d[:])
            nc.sync.dma_start(wo_s[:], wo_d[:])

            # ---- per token block: QKV projections (interleaved into stream)
            def qkv_for_tb(tb):
                xt_t = xtp.tile([128, CT, TBLK], BF16, tag="xt")
                nc.sync.dma_start(xt_t[:], xt_d[:, tb])
                for name, w_s, dstT in (("q", wq_s, qT), ("k", wk_s, kT), ("v", wv_s, None)):
                    ps = psA.tile([128, TBLK], F32, tag="psa", name=f"ps_{name}_{tb}")
                    for ct in range(CT):
                        nc.tensor.matmul(
                            ps[:],
                            w_s[:, ct, :],
                            xt_t[:, ct, :],
                            start=(ct == 0),
                            stop=(ct == CT - 1),
                        )
                    if dstT is not None:
                        nc.vector.tensor_copy(
                            dstT[:, tb * TBLK : (tb + 1) * TBLK], ps[:]
                        )
                    else:
                        vt_t = vtp.tile([128, TBLK], BF16, tag="vt")
                        nc.vector.tensor_copy(vt_t[:], ps[:])
                        # transpose [128(d0|d1), 128k] -> [128k, (V_h0|V_h1)] on PE
                        for sub in range(TBLK // 128):
                            ktg = tb * 4 + sub
                            tps = psA.tile([128, 128], BF16, tag="psa")
                            nc.tensor.transpose(
                                tps[:],
                                vt_t[:, sub * 128 : (sub + 1) * 128],
                                ident[:],
                            )
                            nc.vector.tensor_copy(
                                vN[:, ktg, 1:3, :].rearrange("p a c -> p (a c)"),
                                tps[:],
                            )

            # ---- attention for one (batch, qblock): ktile pairs
            copy_rr = [0]  # round-robin engine picker for outproj copies

            def attn(b, qb):
                t0 = b * T + qb * TBLK  # global token offset of this q block
                O_ps = psO.tile([128, HPC, TBLK], F32, tag="O", name=f"O_{b}_{qb}")
                npair = (qb + 1) * 2
                for kp in range(npair):
                    s0 = kp * 256 - qb * TBLK  # diag offset of first ktile in pair
                    diag = s0 + 128 >= 0
                    rect0 = max(s0, 0)
                    sT = psS.tile([128, 2, HPC, TBLK], F32, tag="sT")
                    es = esp.tile([128, 2, HPC, TBLK], BF16, tag="es")
                    for i in range(2):
                        kt = kp * 2 + i
                        col0 = max(s0 + 128 * i, 0)
                        for h in range(HPC):
                            hp = h * 64
                            nc.tensor.matmul(
                                sT[:, i, h, col0:TBLK],
                                kT[hp : hp + 64, b * T + kt * 128 : b * T + (kt + 1) * 128],
                                qT[hp : hp + 64, t0 + col0 : t0 + TBLK],
                                start=True,
                                stop=True,
                            )
                    nc.scalar.activation(
                        es[:, :, :, rect0:TBLK],
                        sT[:, :, :, rect0:TBLK],
                        mybir.ActivationFunctionType.Exp,
                        scale=0.125,
                    )
                    if diag:
                        # zero everything above the causal diagonal (incl. the
                        # stale-psum region of the second ktile) in one pass:
                        # keep es[p, i, h, col] iff (rect0+col) >= s0+128*i+p
                        nc.gpsimd.affine_select(
                            out=es[:, :, :, rect0 : rect0 + 256],
                            in_=es[:, :, :, rect0 : rect0 + 256],
                            compare_op=mybir.AluOpType.is_ge,
                            fill=0.0,
                            base=rect0 - s0,
                            pattern=[[-128, 2], [0, HPC], [1, 256]],
                            channel_multiplier=-1,
                        )
                    for i in range(2):
                        kt = kp * 2 + i
                        ktg = b * KT + kt
                        col0 = max(s0 + 128 * i, 0)
                        for h in range(HPC):
                            vsta = vN[:, ktg, 0:2, :] if h == 0 else vN[:, ktg, 2:4, :]
                            nc.tensor.matmul(
                                O_ps[:, h, col0:TBLK],
                                vsta,
                                es[:, i, h, col0:TBLK],
                                start=(kp == 0 and i == 0),
                                stop=(kp == npair - 1 and i == 1),
                            )
                # normalize: O / rowsum (rowsum rows: h0 -> 0:64, h1 -> 64:128)
                rs = rinp.tile([128, TBLK], F32, tag="rs")
                rin = rinp.tile([128, TBLK], F32, tag="rin")
                nc.vector.tensor_copy(rs[0:64, :], O_ps[0:64, 0, :])
                nc.vector.tensor_copy(rs[64:128, :], O_ps[64:128, 1, :])
                nc.vector.reciprocal_approx_fast(rin[:], rs[:])
                nc.vector.tensor_mul(
                    oN[0:64, t0 : t0 + TBLK], O_ps[64:128, 0, :], rin[0:64, :]
                )
                nc.vector.tensor_mul(
                    oN[64:128, t0 : t0 + TBLK], O_ps[0:64, 1, :], rin[64:128, :]
                )
                # out-projection for this token block
                tb = b * QB + qb
                ot = outp.tile([128, CT, TBLK], BF16, tag="ot")
                for ct in range(CT):
                    ops = psA.tile([128, TBLK], F32, tag="psa")
                    nc.tensor.matmul(
                        ops[:],
                        wo_s[:, ct * 128 : (ct + 1) * 128],
                        oN[:, t0 : t0 + TBLK],
                        start=True,
                        stop=True,
                    )
                    # gpsimd can't read PSUM; split copies DVE-heavy with some ACT
                    if copy_rr[0] % 4 == 3:
                        nc.scalar.copy(ot[:, ct, :], ops[:])
                    else:
                        nc.vector.tensor_copy(ot[:, ct, :], ops[:])
                    copy_rr[0] += 1
                nc.sync.dma_start(out_d[:, tb], ot[:])

            # ---- interleaved stream: QKV blocks woven between attention units
            qkv_for_tb(0)
            qkv_for_tb(1)
            attn(0, 0)
            qkv_for_tb(2)
            attn(0, 1)
            qkv_for_tb(3)
            attn(0, 2)
            qkv_for_tb(4)
            attn(0, 3)
            qkv_for_tb(5)
            attn(1, 0)
            qkv_for_tb(6)
            attn(1, 1)
            qkv_for_tb(7)
            attn(1, 2)
            attn(1, 3)

    nc.compile()
    return nc


_NC_CACHE = None


def get_program():
    global _NC_CACHE
    if _NC_CACHE is None:
        _NC_CACHE = build_program()
    return _NC_CACHE


def make_in_maps(x, Wq, Wk, Wv, Wo):
    bf = ml_dtypes.bfloat16
    # xt layout [p, tb, ct, t] so each per-tb DMA is 8KB/partition contiguous
    xt = np.asarray(x, np.float32).reshape(NT, C).T  # [C, NT]
    xt = np.ascontiguousarray(
        xt.reshape(CT, 128, NTB, TBLK).transpose(1, 2, 0, 3)
    ).astype(bf)
    wq_b = np.asarray(Wq, np.float32).astype(bf)
    wk_b = np.asarray(Wk, np.float32).astype(bf)
    wv_b = np.asarray(Wv, np.float32).astype(bf)
    wo_b = np.asarray(Wo, np.float32).astype(bf)
    in_maps = []
    for cid in range(N_CORES):
        sl = slice(cid * FPC, (cid + 1) * FPC)
        in_maps.append(
            {
                "xt": xt,
                # [C, fpc] -> [p, ct, fpc]
                "wq": np.ascontiguousarray(
                    wq_b[:, sl].reshape(CT, 128, FPC).transpose(1, 0, 2)
                ),
                "wk": np.ascontiguousarray(
                    wk_b[:, sl].reshape(CT, 128, FPC).transpose(1, 0, 2)
                ),
                "wv": np.ascontiguousarray(
                    wv_b[:, sl].reshape(CT, 128, FPC).transpose(1, 0, 2)
                ),
                "wo": np.ascontiguousarray(wo_b[sl, :]),
            }
        )
    return in_maps


def kernel(x, Wq, Wk, Wv, Wo, bo, _trace=False, _tmpdir=None):
    x = np.asarray(x, dtype=np.float32)
    in_maps = make_in_maps(x, Wq, Wk, Wv, Wo)
    nc = get_program()
    res = run_bass_kernel_spmd(
        nc, in_maps, core_ids=list(range(N_CORES)), trace=_trace, tmpdir=_tmpdir
    )
    acc = res.results[0]["outT"].astype(np.float32)
    for i in range(1, N_CORES):
        acc = acc + res.results[i]["outT"].astype(np.float32)
    # acc [p, tb, ct, t] -> outT [C, NT] with c = ct*128+p, t = tb*512+ti
    outT = acc.transpose(2, 0, 1, 3).reshape(C, NT)
    out = outT.T + np.asarray(bo, np.float32)[None, :]
    if _trace:
        kernel._last_results = res
    return out.reshape(B, T, C).astype(np.float32)


# revision 4
# speedup vs baseline: 1.1706x; 1.1706x over previous
"""Multi-head causal attention (B=2, T=2048, H=16, D=64, C=1024) on 8 trn2 cores.

Sharding: tensor-parallel over heads. Each core owns 2 heads (both batches):
  - computes Q^T/K^T/V^T for its heads over all 4096 tokens
  - causal attention in transposed orientation (S^T[k,q]) so no P transpose
  - partial output projection outT_partial[c, t] = Wo_slice^T @ O^T
Host sums the 8 partials (the "all-reduce"), adds bias, transposes back.

v3 scheduling: the emission order software-pipelines the attention inner loop
(scores of ktile k+1 overlap exp of ktile k via a double-buffered PSUM score
tile) and weaves QKV-projection / output-projection / V-transpose work into
the stream as PE filler so the tensor engine never idles while the scalar
engine computes exp (keeps the HAM clock-gate warm). Causal masking is a
single gpsimd affine_select per diagonal ktile. Partial outputs are written
bf16 with 8KB/partition contiguous DMA layouts; host accumulates in fp32.
"""

import sys

sys.path.insert(0, "/opt/trn_rl_repo")

from collections import deque

import ml_dtypes
import numpy as np

import concourse.bacc as bacc
import concourse.mybir as mybir
import concourse.tile as tile
from concourse.bass_utils import run_bass_kernel_spmd

B, T, C = 2, 2048, 1024
H, D = 16, 64
NT = B * T  # 4096 flattened tokens
N_CORES = 8
HPC = H // N_CORES  # 2 heads per core
FPC = HPC * D  # 128 features per core
CT = C // 128  # 8 contraction tiles for projections
TBLK = 512  # token block
NTB = NT // TBLK  # 8 token blocks
QB = T // TBLK  # 4 query blocks per batch
KT = T // 128  # 16 key tiles per batch

F32 = mybir.dt.float32
BF16 = mybir.dt.bfloat16


def build_program():
    nc = bacc.Bacc("TRN2", target_bir_lowering=False, debug=False)

    xt_d = nc.declare_dram_parameter("xt", [128, NTB, CT, TBLK], BF16, isOutput=False)
    wq_d = nc.declare_dram_parameter("wq", [128, CT, FPC], BF16, isOutput=False)
    wk_d = nc.declare_dram_parameter("wk", [128, CT, FPC], BF16, isOutput=False)
    wv_d = nc.declare_dram_parameter("wv", [128, CT, FPC], BF16, isOutput=False)
    wo_d = nc.declare_dram_parameter("wo", [FPC, C], BF16, isOutput=False)
    out_d = nc.declare_dram_parameter("outT", [128, NTB, CT, TBLK], BF16, isOutput=True)

    with tile.TileContext(nc) as tc:
        with (
            tc.tile_pool(name="slabs", bufs=1) as slabs,
            tc.tile_pool(name="xtp", bufs=2) as xtp,
            tc.tile_pool(name="esp", bufs=3) as esp,
            tc.tile_pool(name="vtp", bufs=2) as vtp,
            tc.tile_pool(name="rinp", bufs=2) as rinp,
            tc.tile_pool(name="outp", bufs=2) as outp,
            tc.tile_pool(name="psS", bufs=2, space="PSUM") as psS,  # 2x2 banks
            tc.tile_pool(name="psO", bufs=1, space="PSUM") as psO,  # 2 banks
            tc.tile_pool(name="psQ", bufs=1, space="PSUM") as psQ,  # 1 bank
            tc.tile_pool(name="psA", bufs=1, space="PSUM") as psA,  # 1 bank
        ):
            # ---- persistent slabs
            qT = slabs.tile([128, NT], BF16, tag="qT")  # [2h*64d, t]
            kT = slabs.tile([128, NT], BF16, tag="kT")
            # V natural layout: per ktile_global: [128k, (ones | V_h0 | V_h1 | ones)]
            # PV stationary h0 = [:, ktg, 0:2, :] = [ones|V_h0] -> rowsum rows 0:64, O 64:128
            #               h1 = [:, ktg, 2:4, :] = [V_h1|ones] -> O rows 0:64, rowsum 64:128
            vN = slabs.tile([128, NTB * 4, 4, 64], BF16, tag="vN")
            oN = slabs.tile([128, NT], BF16, tag="oN")  # normalized O^T
            wq_s = slabs.tile([128, CT, FPC], BF16, tag="wq")
            wk_s = slabs.tile([128, CT, FPC], BF16, tag="wk")
            wv_s = slabs.tile([128, CT, FPC], BF16, tag="wv")
            wo_s = slabs.tile([128, C], BF16, tag="wo")  # [f, c]
            ident = slabs.tile([128, 128], BF16, tag="ident")

            # ---- constants
            from concourse.masks import make_identity
            make_identity(nc, ident[:])
            nc.gpsimd.memset(vN[:, :, 0, :], 1.0)
            nc.gpsimd.memset(vN[:, :, 3, :], 1.0)

            # ---- weight loads
            nc.sync.dma_start(wq_s[:], wq_d[:])
            nc.sync.dma_start(wk_s[:], wk_d[:])
            nc.sync.dma_start(wv_s[:], wv_d[:])
            nc.sync.dma_start(wo_s[:], wo_d[:])

            # ---- filler work queue: each entry is a (qkv_tb_watermark, fn)
            filler = deque()
            qkv_done = [-1]  # highest tb fully emitted

            def emit_filler(n):
                for _ in range(n):
                    if not filler:
                        return
                    tb_mark, fn = filler.popleft()
                    fn()
                    qkv_done[0] = max(qkv_done[0], tb_mark)

            def drain_filler_until(tb_needed):
                while qkv_done[0] < tb_needed and filler:
                    emit_filler(1)

            # ---- QKV projection steps for one token block (queued as filler)
            def queue_qkv(tb):
                xt_t = xtp.tile([128, CT, TBLK], BF16, tag="xt", name=f"xt_{tb}")
                state = {}

                def dma_step():
                    nc.sync.dma_start(xt_t[:], xt_d[:, tb])

                filler.append((-1, dma_step))

                def mm_step(name, w_s, ct0, nct):
                    def fn():
                        if ct0 == 0:
                            state[name] = psQ.tile(
                                [128, TBLK], F32, tag="psq", name=f"ps_{name}_{tb}"
                            )
                        ps = state[name]
                        for ct in range(ct0, ct0 + nct):
                            nc.tensor.matmul(
                                ps[:],
                                w_s[:, ct, :],
                                xt_t[:, ct, :],
                                start=(ct == 0),
                                stop=(ct == CT - 1),
                            )
                    return fn

                def cast_step(name, dstT):
                    def fn():
                        nc.vector.tensor_copy(
                            dstT[:, tb * TBLK : (tb + 1) * TBLK], state[name][:]
                        )
                    return fn

                for name, w_s, dstT in (("q", wq_s, qT), ("k", wk_s, kT)):
                    filler.append((-1, mm_step(name, w_s, 0, 4)))
                    filler.append((-1, mm_step(name, w_s, 4, 4)))
                    filler.append((-1, cast_step(name, dstT)))
                filler.append((-1, mm_step("v", wv_s, 0, 4)))
                filler.append((-1, mm_step("v", wv_s, 4, 4)))

                vt_t = vtp.tile([128, TBLK], BF16, tag="vt", name=f"vt_{tb}")

                def vcast_step():
                    nc.vector.tensor_copy(vt_t[:], state["v"][:])

                filler.append((-1, vcast_step))

                def trans_step(sub, mark):
                    def fn():
                        ktg = tb * 4 + sub
                        tps = psA.tile([128, 128], BF16, tag="psa")
                        nc.tensor.transpose(
                            tps[:], vt_t[:, sub * 128 : (sub + 1) * 128], ident[:]
                        )
                        nc.vector.tensor_copy(
                            vN[:, ktg, 1:3, :].rearrange("p a c -> p (a c)"), tps[:]
                        )
                    return fn

                for sub in range(4):
                    filler.append((tb if sub == 3 else -1, trans_step(sub, tb)))

            # ---- output projection steps for one attention unit (queued as filler)
            copy_rr = [0]

            def queue_outproj(b, qb):
                t0 = b * T + qb * TBLK
                tb = b * QB + qb
                ot = outp.tile([128, CT, TBLK], BF16, tag="ot", name=f"ot_{tb}")

                def proj_step(ct):
                    def fn():
                        ops = psA.tile([128, TBLK], F32, tag="psa")
                        nc.tensor.matmul(
                            ops[:],
                            wo_s[:, ct * 128 : (ct + 1) * 128],
                            oN[:, t0 : t0 + TBLK],
                            start=True,
                            stop=True,
                        )
                        if copy_rr[0] % 4 == 3:
                            nc.scalar.copy(ot[:, ct, :], ops[:])
                        else:
                            nc.vector.tensor_copy(ot[:, ct, :], ops[:])
                        copy_rr[0] += 1
                        if ct == CT - 1:
                            nc.sync.dma_start(out_d[:, tb], ot[:])
                    return fn

                for ct in range(CT):
                    filler.append((-1, proj_step(ct)))

            # ---- attention for one (batch, qblock), software-pipelined
            def attn(b, qb):
                drain_filler_until(b * QB + qb)
                t0 = b * T + qb * TBLK
                O_ps = psO.tile([128, HPC, TBLK], F32, tag="O", name=f"O_{b}_{qb}")
                nkt = (qb + 1) * 4
                prev = None

                def scores_exp(kt):
                    s = kt * 128 - qb * TBLK
                    col0 = max(s, 0)
                    sT = psS.tile([128, HPC, TBLK], F32, tag="sT")
                    es = esp.tile([128, HPC, TBLK], BF16, tag="es")
                    for h in range(HPC):
                        hp = h * 64
                        nc.tensor.matmul(
                            sT[:, h, col0:TBLK],
                            kT[hp : hp + 64, b * T + kt * 128 : b * T + (kt + 1) * 128],
                            qT[hp : hp + 64, t0 + col0 : t0 + TBLK],
                            start=True,
                            stop=True,
                        )
                    nc.scalar.activation(
                        es[:, :, col0:TBLK],
                        sT[:, :, col0:TBLK],
                        mybir.ActivationFunctionType.Exp,
                        scale=0.125,
                    )
                    if s >= 0:
                        # zero strictly-above-diagonal: keep es[p,h,col] iff col>=p
                        nc.gpsimd.affine_select(
                            out=es[:, :, col0 : col0 + 128],
                            in_=es[:, :, col0 : col0 + 128],
                            compare_op=mybir.AluOpType.is_ge,
                            fill=0.0,
                            base=0,
                            pattern=[[0, HPC], [1, 128]],
                            channel_multiplier=-1,
                        )
                    return es, col0

                def pv(kt, es, col0):
                    ktg = b * KT + kt
                    for h in range(HPC):
                        vsta = vN[:, ktg, 0:2, :] if h == 0 else vN[:, ktg, 2:4, :]
                        nc.tensor.matmul(
                            O_ps[:, h, col0:TBLK],
                            vsta,
                            es[:, h, col0:TBLK],
                            start=(kt == 0),
                            stop=(kt == nkt - 1),
                        )

                for kt in range(nkt):
                    cur = (kt, *scores_exp(kt))
                    if prev is not None:
                        pv(*prev)
                    prev = cur
                    emit_filler(2)
                pv(*prev)

                # normalize: O / rowsum (rowsum rows: h0 -> 0:64, h1 -> 64:128)
                rs = rinp.tile([128, TBLK], F32, tag="rs")
                rin = rinp.tile([128, TBLK], F32, tag="rin")
                nc.vector.tensor_copy(rs[0:64, :], O_ps[0:64, 0, :])
                nc.vector.tensor_copy(rs[64:128, :], O_ps[64:128, 1, :])
                nc.vector.reciprocal_approx_fast(rin[:], rs[:])
                nc.vector.tensor_mul(
                    oN[0:64, t0 : t0 + TBLK], O_ps[64:128, 0, :], rin[0:64, :]
                )
                nc.vector.tensor_mul(
                    oN[64:128, t0 : t0 + TBLK], O_ps[0:64, 1, :], rin[64:128, :]
                )
                queue_outproj(b, qb)

            # ---- the stream
            for tb in range(NTB):
                queue_qkv(tb)
            # prologue: get qkv(0) fully emitted before first attention
            drain_filler_until(0)
            for b, qb in (
                (0, 0), (0, 1), (0, 2), (0, 3),
                (1, 3), (1, 2), (1, 1), (1, 0),
            ):
                attn(b, qb)
            emit_filler(10**9)

    nc.compile()
    return nc


_NC_CACHE = None


def get_program():
    global _NC_CACHE
    if _NC_CACHE is None:
        _NC_CACHE = build_program()
    return _NC_CACHE


def make_in_maps(x, Wq, Wk, Wv, Wo):
    bf = ml_dtypes.bfloat16
    # xt layout [p, tb, ct, t] so each per-tb DMA is 8KB/partition contiguous
    xt = np.asarray(x, np.float32).reshape(NT, C).T  # [C, NT]
    xt = np.ascontiguousarray(
        xt.reshape(CT, 128, NTB, TBLK).transpose(1, 2, 0, 3)
    ).astype(bf)
    wq_b = np.asarray(Wq, np.float32).astype(bf)
    wk_b = np.asarray(Wk, np.float32).astype(bf)
    wv_b = np.asarray(Wv, np.float32).astype(bf)
    wo_b = np.asarray(Wo, np.float32).astype(bf)
    in_maps = []
    for cid in range(N_CORES):
        sl = slice(cid * FPC, (cid + 1) * FPC)
        in_maps.append(
            {
                "xt": xt,
                "wq": np.ascontiguousarray(
                    wq_b[:, sl].reshape(CT, 128, FPC).transpose(1, 0, 2)
                ),
                "wk": np.ascontiguousarray(
                    wk_b[:, sl].reshape(CT, 128, FPC).transpose(1, 0, 2)
                ),
                "wv": np.ascontiguousarray(
                    wv_b[:, sl].reshape(CT, 128, FPC).transpose(1, 0, 2)
                ),
                "wo": np.ascontiguousarray(wo_b[sl, :]),
            }
        )
    return in_maps


def kernel(x, Wq, Wk, Wv, Wo, bo, _trace=False, _tmpdir=None):
    x = np.asarray(x, dtype=np.float32)
    in_maps = make_in_maps(x, Wq, Wk, Wv, Wo)
    nc = get_program()
    res = run_bass_kernel_spmd(
        nc, in_maps, core_ids=list(range(N_CORES)), trace=_trace, tmpdir=_tmpdir
    )
    acc = res.results[0]["outT"].astype(np.float32)
    for i in range(1, N_CORES):
        acc = acc + res.results[i]["outT"].astype(np.float32)
    # acc [p, tb, ct, t] -> outT [C, NT] with c = ct*128+p, t = tb*512+ti
    outT = acc.transpose(2, 0, 1, 3).reshape(C, NT)
    out = outT.T + np.asarray(bo, np.float32)[None, :]
    if _trace:
        kernel._last_results = res
    return out.reshape(B, T, C).astype(np.float32)


# revision 6
# speedup vs baseline: 1.3380x; 1.1430x over previous
"""Multi-head causal attention (B=2, T=2048, H=16, D=64, C=1024) on 8 trn2 cores.

Sharding: tensor-parallel over heads. Each core owns 2 heads (both batches):
  - computes Q^T/K^T/V^T for its heads over all 4096 tokens
  - causal attention in transposed orientation (S^T[k,q]) so no P transpose
  - partial output projection outT_partial[c, t] = Wo_slice^T @ O^T
Host sums the 8 partials (the "all-reduce"), adds bias, transposes back.

v3 scheduling: the emission order software-pipelines the attention inner loop
(scores of ktile k+1 overlap exp of ktile k via a double-buffered PSUM score
tile) and weaves QKV-projection / output-projection / V-transpose work into
the stream as PE filler so the tensor engine never idles while the scalar
engine computes exp (keeps the HAM clock-gate warm). Causal masking is a
single gpsimd affine_select per diagonal ktile. Partial outputs are written
bf16 with 8KB/partition contiguous DMA layouts; host accumulates in fp32.
"""

import sys

sys.path.insert(0, "/opt/trn_rl_repo")

from collections import deque

import ml_dtypes
import numpy as np

import concourse.bacc as bacc
import concourse.mybir as mybir
import concourse.tile as tile
from concourse.bass_utils import run_bass_kernel_spmd

B, T, C = 2, 2048, 1024
H, D = 16, 64
NT = B * T  # 4096 flattened tokens
N_CORES = 8
HPC = H // N_CORES  # 2 heads per core
FPC = HPC * D  # 128 features per core
CT = C // 128  # 8 contraction tiles for projections
TBLK = 512  # token block
NTB = NT // TBLK  # 8 token blocks
QB = T // TBLK  # 4 query blocks per batch
KT = T // 128  # 16 key tiles per batch

F32 = mybir.dt.float32
BF16 = mybir.dt.bfloat16


def build_program():
    nc = bacc.Bacc("TRN2", target_bir_lowering=False, debug=False)

    xt_d = nc.declare_dram_parameter("xt", [128, NTB, CT, TBLK], BF16, isOutput=False)
    wq_d = nc.declare_dram_parameter("wq", [128, CT, FPC], BF16, isOutput=False)
    wk_d = nc.declare_dram_parameter("wk", [128, CT, FPC], BF16, isOutput=False)
    wv_d = nc.declare_dram_parameter("wv", [128, CT, FPC], BF16, isOutput=False)
    wo_d = nc.declare_dram_parameter("wo", [FPC, C], BF16, isOutput=False)
    out_d = nc.declare_dram_parameter("outT", [128, NTB, CT, TBLK], BF16, isOutput=True)

    with tile.TileContext(nc) as tc:
        with (
            tc.tile_pool(name="slabs", bufs=1) as slabs,
            tc.tile_pool(name="xtp", bufs=2) as xtp,
            tc.tile_pool(name="esp", bufs=3) as esp,
            tc.tile_pool(name="vtp", bufs=2) as vtp,
            tc.tile_pool(name="rinp", bufs=2) as rinp,
            tc.tile_pool(name="outp", bufs=2) as outp,
            tc.tile_pool(name="psS", bufs=2, space="PSUM") as psS,  # 2x2 banks
            tc.tile_pool(name="psO", bufs=1, space="PSUM") as psO,  # 2 banks
            tc.tile_pool(name="psW", bufs=2, space="PSUM") as psW,  # 2x1 bank
        ):
            # ---- persistent slabs
            qT = slabs.tile([128, NT], BF16, tag="qT")  # [2h*64d, t]
            kT = slabs.tile([128, NT], BF16, tag="kT")
            # V natural layout: per ktile_global: [128k, (ones | V_h0 | V_h1 | ones)]
            # PV stationary h0 = [:, ktg, 0:2, :] = [ones|V_h0] -> rowsum rows 0:64, O 64:128
            #               h1 = [:, ktg, 2:4, :] = [V_h1|ones] -> O rows 0:64, rowsum 64:128
            vN = slabs.tile([128, NTB * 4, 4, 64], BF16, tag="vN")
            oN = slabs.tile([128, NT], BF16, tag="oN")  # normalized O^T
            wq_s = slabs.tile([128, CT, FPC], BF16, tag="wq")
            wk_s = slabs.tile([128, CT, FPC], BF16, tag="wk")
            wv_s = slabs.tile([128, CT, FPC], BF16, tag="wv")
            wo_s = slabs.tile([128, C], BF16, tag="wo")  # [f, c]
            ident = slabs.tile([128, 128], BF16, tag="ident")

            # ---- constants
            from concourse.masks import make_identity
            make_identity(nc, ident[:])
            nc.gpsimd.memset(vN[:, :, 0, :], 1.0)
            nc.gpsimd.memset(vN[:, :, 3, :], 1.0)

            # ---- weight loads
            nc.sync.dma_start(wq_s[:], wq_d[:])
            nc.sync.dma_start(wk_s[:], wk_d[:])
            nc.sync.dma_start(wv_s[:], wv_d[:])
            nc.sync.dma_start(wo_s[:], wo_d[:])

            # ---- filler work queue: each entry is a (qkv_tb_watermark, fn)
            filler = deque()
            qkv_done = [-1]  # highest tb fully emitted

            def emit_filler(n):
                for _ in range(n):
                    if not filler:
                        return
                    tb_mark, fn = filler.popleft()
                    fn()
                    qkv_done[0] = max(qkv_done[0], tb_mark)

            def drain_filler_until(tb_needed):
                while qkv_done[0] < tb_needed and filler:
                    emit_filler(1)

            # ---- QKV projection steps for one token block (queued as filler)
            def queue_qkv(tb):
                xt_t = xtp.tile([128, CT, TBLK], BF16, tag="xt", name=f"xt_{tb}")
                state = {}

                def dma_step():
                    nc.sync.dma_start(xt_t[:], xt_d[:, tb])

                filler.append((-1, dma_step))

                def mm_step(name, w_s, ct0, nct):
                    def fn():
                        if ct0 == 0:
                            state[name] = psW.tile(
                                [128, TBLK], F32, tag="psw", name=f"ps_{name}_{tb}"
                            )
                        ps = state[name]
                        for ct in range(ct0, ct0 + nct):
                            nc.tensor.matmul(
                                ps[:],
                                w_s[:, ct, :],
                                xt_t[:, ct, :],
                                start=(ct == 0),
                                stop=(ct == CT - 1),
                            )
                    return fn

                def cast_step(name, dstT):
                    def fn():
                        nc.vector.tensor_copy(
                            dstT[:, tb * TBLK : (tb + 1) * TBLK], state[name][:]
                        )
                    return fn

                for name, w_s, dstT in (("q", wq_s, qT), ("k", wk_s, kT)):
                    filler.append((-1, mm_step(name, w_s, 0, 4)))
                    filler.append((-1, mm_step(name, w_s, 4, 4)))
                    filler.append((-1, cast_step(name, dstT)))
                filler.append((-1, mm_step("v", wv_s, 0, 4)))
                filler.append((-1, mm_step("v", wv_s, 4, 4)))

                vt_t = vtp.tile([128, TBLK], BF16, tag="vt", name=f"vt_{tb}")

                def vcast_step():
                    nc.vector.tensor_copy(vt_t[:], state["v"][:])

                filler.append((-1, vcast_step))

                def trans_step(sub, mark):
                    def fn():
                        ktg = tb * 4 + sub
                        tps = psW.tile([128, 128], BF16, tag="psw")
                        nc.tensor.transpose(
                            tps[:], vt_t[:, sub * 128 : (sub + 1) * 128], ident[:]
                        )
                        nc.vector.tensor_copy(
                            vN[:, ktg, 1:3, :].rearrange("p a c -> p (a c)"), tps[:]
                        )
                    return fn

                for sub in range(4):
                    filler.append((tb if sub == 3 else -1, trans_step(sub, tb)))

            # ---- output projection steps for one attention unit (queued as filler)
            copy_rr = [0]

            def queue_outproj(b, qb):
                t0 = b * T + qb * TBLK
                tb = b * QB + qb
                ot = outp.tile([128, CT, TBLK], BF16, tag="ot", name=f"ot_{tb}")

                def proj_step(ct):
                    def fn():
                        ops = psW.tile([128, TBLK], F32, tag="psw")
                        nc.tensor.matmul(
                            ops[:],
                            wo_s[:, ct * 128 : (ct + 1) * 128],
                            oN[:, t0 : t0 + TBLK],
                            start=True,
                            stop=True,
                        )
                        if copy_rr[0] % 4 == 3:
                            nc.scalar.copy(ot[:, ct, :], ops[:])
                        else:
                            nc.vector.tensor_copy(ot[:, ct, :], ops[:])
                        copy_rr[0] += 1
                        if ct == CT - 1:
                            nc.sync.dma_start(out_d[:, tb], ot[:])
                    return fn

                for ct in range(CT):
                    filler.append((-1, proj_step(ct)))

            # ---- attention for one (batch, qblock), software-pipelined
            def attn(b, qb):
                drain_filler_until(b * QB + qb)
                t0 = b * T + qb * TBLK
                O_ps = psO.tile([128, HPC, TBLK], F32, tag="O", name=f"O_{b}_{qb}")
                nkt = (qb + 1) * 4
                prev = None

                def scores_exp(kt):
                    s = kt * 128 - qb * TBLK
                    col0 = max(s, 0)
                    sT = psS.tile([128, HPC, TBLK], F32, tag="sT")
                    es = esp.tile([128, HPC, TBLK], BF16, tag="es")
                    for h in range(HPC):
                        hp = h * 64
                        nc.tensor.matmul(
                            sT[:, h, col0:TBLK],
                            kT[hp : hp + 64, b * T + kt * 128 : b * T + (kt + 1) * 128],
                            qT[hp : hp + 64, t0 + col0 : t0 + TBLK],
                            start=True,
                            stop=True,
                        )
                    nc.scalar.activation(
                        es[:, :, col0:TBLK],
                        sT[:, :, col0:TBLK],
                        mybir.ActivationFunctionType.Exp,
                        scale=0.125,
                    )
                    if s >= 0:
                        # zero strictly-above-diagonal: keep es[p,h,col] iff col>=p
                        nc.gpsimd.affine_select(
                            out=es[:, :, col0 : col0 + 128],
                            in_=es[:, :, col0 : col0 + 128],
                            compare_op=mybir.AluOpType.is_ge,
                            fill=0.0,
                            base=0,
                            pattern=[[0, HPC], [1, 128]],
                            channel_multiplier=-1,
                        )
                    return es, col0

                def pv(kt, es, col0):
                    ktg = b * KT + kt
                    for h in range(HPC):
                        vsta = vN[:, ktg, 0:2, :] if h == 0 else vN[:, ktg, 2:4, :]
                        nc.tensor.matmul(
                            O_ps[:, h, col0:TBLK],
                            vsta,
                            es[:, h, col0:TBLK],
                            start=(kt == 0),
                            stop=(kt == nkt - 1),
                        )

                for kt in range(nkt):
                    cur = (kt, *scores_exp(kt))
                    if prev is not None:
                        pv(*prev)
                    prev = cur
                    emit_filler(2)
                pv(*prev)

                # normalize: O / rowsum (rowsum rows: h0 -> 0:64, h1 -> 64:128)
                rs = rinp.tile([128, TBLK], F32, tag="rs")
                rin = rinp.tile([128, TBLK], F32, tag="rin")
                nc.vector.tensor_copy(rs[0:64, :], O_ps[0:64, 0, :])
                nc.vector.tensor_copy(rs[64:128, :], O_ps[64:128, 1, :])
                nc.vector.reciprocal_approx_fast(rin[:], rs[:])
                nc.vector.tensor_mul(
                    oN[0:64, t0 : t0 + TBLK], O_ps[64:128, 0, :], rin[0:64, :]
                )
                nc.vector.tensor_mul(
                    oN[64:128, t0 : t0 + TBLK], O_ps[0:64, 1, :], rin[64:128, :]
                )
                queue_outproj(b, qb)

            # ---- the stream
            for tb in range(NTB):
                queue_qkv(tb)
            # prologue: get qkv(0) fully emitted before first attention
            drain_filler_until(0)
            for b, qb in (
                (0, 0), (0, 1), (0, 2), (0, 3),
                (1, 1), (1, 2), (1, 3), (1, 0),
            ):
                attn(b, qb)
            emit_filler(10**9)

    nc.compile()
    return nc


_NC_CACHE = None


def get_program():
    global _NC_CACHE
    if _NC_CACHE is None:
        _NC_CACHE = build_program()
    return _NC_CACHE


def make_in_maps(x, Wq, Wk, Wv, Wo):
    bf = ml_dtypes.bfloat16
    # xt layout [p, tb, ct, t] so each per-tb DMA is 8KB/partition contiguous
    xt = np.asarray(x, np.float32).reshape(NT, C).T  # [C, NT]
    xt = np.ascontiguousarray(
        xt.reshape(CT, 128, NTB, TBLK).transpose(1, 2, 0, 3)
    ).astype(bf)
    wq_b = np.asarray(Wq, np.float32).astype(bf)
    wk_b = np.asarray(Wk, np.float32).astype(bf)
    wv_b = np.asarray(Wv, np.float32).astype(bf)
    wo_b = np.asarray(Wo, np.float32).astype(bf)
    in_maps = []
    for cid in range(N_CORES):
        sl = slice(cid * FPC, (cid + 1) * FPC)
        in_maps.append(
            {
                "xt": xt,
                "wq": np.ascontiguousarray(
                    wq_b[:, sl].reshape(CT, 128, FPC).transpose(1, 0, 2)
                ),
                "wk": np.ascontiguousarray(
                    wk_b[:, sl].reshape(CT, 128, FPC).transpose(1, 0, 2)
                ),
                "wv": np.ascontiguousarray(
                    wv_b[:, sl].reshape(CT, 128, FPC).transpose(1, 0, 2)
                ),
                "wo": np.ascontiguousarray(wo_b[sl, :]),
            }
        )
    return in_maps


def kernel(x, Wq, Wk, Wv, Wo, bo, _trace=False, _tmpdir=None):
    x = np.asarray(x, dtype=np.float32)
    in_maps = make_in_maps(x, Wq, Wk, Wv, Wo)
    nc = get_program()
    res = run_bass_kernel_spmd(
        nc, in_maps, core_ids=list(range(N_CORES)), trace=_trace, tmpdir=_tmpdir
    )
    acc = res.results[0]["outT"].astype(np.float32)
    for i in range(1, N_CORES):
        acc = acc + res.results[i]["outT"].astype(np.float32)
    # acc [p, tb, ct, t] -> outT [C, NT] with c = ct*128+p, t = tb*512+ti
    outT = acc.transpose(2, 0, 1, 3).reshape(C, NT)
    out = outT.T + np.asarray(bo, np.float32)[None, :]
    if _trace:
        kernel._last_results = res
    return out.reshape(B, T, C).astype(np.float32)


# revision 7
# speedup vs baseline: 1.3562x; 1.0136x over previous
"""Multi-head causal attention (B=2, T=2048, H=16, D=64, C=1024) on 8 trn2 cores.

Sharding: tensor-parallel over heads. Each core owns 2 heads (both batches):
  - computes Q^T/K^T/V^T for its heads over all 4096 tokens
  - causal attention in transposed orientation (S^T[k,q]) so no P transpose
  - partial output projection outT_partial[c, t] = Wo_slice^T @ O^T
Host sums the 8 partials (the "all-reduce"), adds bias, transposes back.

v3 scheduling: the emission order software-pipelines the attention inner loop
(scores of ktile k+1 overlap exp of ktile k via a double-buffered PSUM score
tile) and weaves QKV-projection / output-projection / V-transpose work into
the stream as PE filler so the tensor engine never idles while the scalar
engine computes exp (keeps the HAM clock-gate warm). Causal masking is a
single gpsimd affine_select per diagonal ktile. Partial outputs are written
bf16 with 8KB/partition contiguous DMA layouts; host accumulates in fp32.
"""

import sys

sys.path.insert(0, "/opt/trn_rl_repo")

from collections import deque

import ml_dtypes
import numpy as np

import concourse.bacc as bacc
import concourse.mybir as mybir
import concourse.tile as tile
from concourse.bass_utils import run_bass_kernel_spmd

B, T, C = 2, 2048, 1024
H, D = 16, 64
NT = B * T  # 4096 flattened tokens
N_CORES = 8
HPC = H // N_CORES  # 2 heads per core
FPC = HPC * D  # 128 features per core
CT = C // 128  # 8 contraction tiles for projections
TBLK = 512  # token block
NTB = NT // TBLK  # 8 token blocks
QB = T // TBLK  # 4 query blocks per batch
KT = T // 128  # 16 key tiles per batch

F32 = mybir.dt.float32
BF16 = mybir.dt.bfloat16


def build_program():
    nc = bacc.Bacc("TRN2", target_bir_lowering=False, debug=False)

    xt_d = nc.declare_dram_parameter("xt", [128, NTB, CT, TBLK], BF16, isOutput=False)
    wq_d = nc.declare_dram_parameter("wq", [128, CT, FPC], BF16, isOutput=False)
    wk_d = nc.declare_dram_parameter("wk", [128, CT, FPC], BF16, isOutput=False)
    wv_d = nc.declare_dram_parameter("wv", [128, CT, FPC], BF16, isOutput=False)
    wo_d = nc.declare_dram_parameter("wo", [FPC, C], BF16, isOutput=False)
    out_d = nc.declare_dram_parameter("outT", [128, NTB, CT, TBLK], BF16, isOutput=True)

    with tile.TileContext(nc) as tc:
        with (
            tc.tile_pool(name="slabs", bufs=1) as slabs,
            tc.tile_pool(name="xtp", bufs=2) as xtp,
            tc.tile_pool(name="esp", bufs=3) as esp,
            tc.tile_pool(name="vtp", bufs=2) as vtp,
            tc.tile_pool(name="rinp", bufs=2) as rinp,
            tc.tile_pool(name="outp", bufs=2) as outp,
            tc.tile_pool(name="psS", bufs=2, space="PSUM") as psS,  # 2x2 banks
            tc.tile_pool(name="psO", bufs=1, space="PSUM") as psO,  # 2 banks
            tc.tile_pool(name="psW", bufs=2, space="PSUM") as psW,  # 2x1 bank
        ):
            # ---- persistent slabs
            qT = slabs.tile([128, NT], BF16, tag="qT")  # [2h*64d, t]
            kT = slabs.tile([128, NT], BF16, tag="kT")
            # V natural layout: per ktile_global: [128k, (ones | V_h0 | V_h1 | ones)]
            # PV stationary h0 = [:, ktg, 0:2, :] = [ones|V_h0] -> rowsum rows 0:64, O 64:128
            #               h1 = [:, ktg, 2:4, :] = [V_h1|ones] -> O rows 0:64, rowsum 64:128
            vN = slabs.tile([128, NTB * 4, 4, 64], BF16, tag="vN")
            oN = slabs.tile([128, NT], BF16, tag="oN")  # normalized O^T
            wq_s = slabs.tile([128, CT, FPC], BF16, tag="wq")
            wk_s = slabs.tile([128, CT, FPC], BF16, tag="wk")
            wv_s = slabs.tile([128, CT, FPC], BF16, tag="wv")
            wo_s = slabs.tile([128, C], BF16, tag="wo")  # [f, c]
            ident = slabs.tile([128, 128], BF16, tag="ident")

            # ---- constants
            from concourse.masks import make_identity
            make_identity(nc, ident[:])
            nc.gpsimd.memset(vN[:, :, 0, :], 1.0)
            nc.gpsimd.memset(vN[:, :, 3, :], 1.0)

            # ---- weight loads
            nc.sync.dma_start(wq_s[:], wq_d[:])
            nc.sync.dma_start(wk_s[:], wk_d[:])
            nc.sync.dma_start(wv_s[:], wv_d[:])
            nc.sync.dma_start(wo_s[:], wo_d[:])

            # ---- filler work queue: each entry is a (qkv_tb_watermark, fn)
            filler = deque()
            qkv_done = [-1]  # highest tb fully emitted

            def emit_filler(n):
                for _ in range(n):
                    if not filler:
                        return
                    tb_mark, fn = filler.popleft()
                    fn()
                    qkv_done[0] = max(qkv_done[0], tb_mark)

            def drain_filler_until(tb_needed):
                while qkv_done[0] < tb_needed and filler:
                    emit_filler(1)

            # ---- QKV projection steps for one token block (queued as filler)
            def queue_qkv(tb):
                xt_t = xtp.tile([128, CT, TBLK], BF16, tag="xt", name=f"xt_{tb}")
                state = {}

                def dma_step():
                    if tb == 0:
                        nc.sync.dma_start(xt_t[:, 0:4], xt_d[:, tb, 0:4])
                        nc.sync.dma_start(xt_t[:, 4:8], xt_d[:, tb, 4:8])
                    else:
                        nc.sync.dma_start(xt_t[:], xt_d[:, tb])

                filler.append((-1, dma_step))

                def mm_step(name, w_s, ct0, nct):
                    def fn():
                        if ct0 == 0:
                            state[name] = psW.tile(
                                [128, TBLK], F32, tag="psw", name=f"ps_{name}_{tb}"
                            )
                        ps = state[name]
                        for ct in range(ct0, ct0 + nct):
                            nc.tensor.matmul(
                                ps[:],
                                w_s[:, ct, :],
                                xt_t[:, ct, :],
                                start=(ct == 0),
                                stop=(ct == CT - 1),
                            )
                    return fn

                def cast_step(name, dstT):
                    def fn():
                        nc.vector.tensor_copy(
                            dstT[:, tb * TBLK : (tb + 1) * TBLK], state[name][:]
                        )
                    return fn

                for name, w_s, dstT in (("q", wq_s, qT), ("k", wk_s, kT)):
                    filler.append((-1, mm_step(name, w_s, 0, 4)))
                    filler.append((-1, mm_step(name, w_s, 4, 4)))
                    filler.append((-1, cast_step(name, dstT)))
                filler.append((-1, mm_step("v", wv_s, 0, 4)))
                filler.append((-1, mm_step("v", wv_s, 4, 4)))

                vt_t = vtp.tile([128, TBLK], BF16, tag="vt", name=f"vt_{tb}")

                def vcast_step():
                    nc.vector.tensor_copy(vt_t[:], state["v"][:])

                filler.append((-1, vcast_step))

                def trans_step(sub, mark):
                    def fn():
                        ktg = tb * 4 + sub
                        tps = psW.tile([128, 128], BF16, tag="psw")
                        nc.tensor.transpose(
                            tps[:], vt_t[:, sub * 128 : (sub + 1) * 128], ident[:]
                        )
                        nc.vector.tensor_copy(
                            vN[:, ktg, 1:3, :].rearrange("p a c -> p (a c)"), tps[:]
                        )
                    return fn

                for sub in range(4):
                    filler.append((tb if sub == 3 else -1, trans_step(sub, tb)))

            # ---- output projection steps for one attention unit (queued as filler)
            copy_rr = [0]

            def queue_outproj(b, qb, spread=False):
                t0 = b * T + qb * TBLK
                tb = b * QB + qb
                ot = outp.tile([128, CT, TBLK], BF16, tag="ot", name=f"ot_{tb}")

                def proj_step(ct):
                    def fn():
                        ops = psW.tile([128, TBLK], F32, tag="psw")
                        nc.tensor.matmul(
                            ops[:],
                            wo_s[:, ct * 128 : (ct + 1) * 128],
                            oN[:, t0 : t0 + TBLK],
                            start=True,
                            stop=True,
                        )
                        on_scalar = (ct % 2 == 1) if spread else (copy_rr[0] % 4 == 3)
                        if on_scalar:
                            nc.scalar.copy(ot[:, ct, :], ops[:])
                        else:
                            nc.vector.tensor_copy(ot[:, ct, :], ops[:])
                        copy_rr[0] += 1
                        if ct == 3:
                            nc.sync.dma_start(out_d[:, tb, 0:4], ot[:, 0:4])
                        elif ct == CT - 1:
                            nc.sync.dma_start(out_d[:, tb, 4:8], ot[:, 4:8])
                    return fn

                for ct in range(CT):
                    filler.append((-1, proj_step(ct)))

            # ---- attention for one (batch, qblock), software-pipelined
            def attn(b, qb):
                drain_filler_until(b * QB + qb)
                t0 = b * T + qb * TBLK
                O_ps = psO.tile([128, HPC, TBLK], F32, tag="O", name=f"O_{b}_{qb}")
                nkt = (qb + 1) * 4
                prev = None

                def scores_exp(kt):
                    s = kt * 128 - qb * TBLK
                    col0 = max(s, 0)
                    sT = psS.tile([128, HPC, TBLK], F32, tag="sT")
                    es = esp.tile([128, HPC, TBLK], BF16, tag="es")
                    for h in range(HPC):
                        hp = h * 64
                        nc.tensor.matmul(
                            sT[:, h, col0:TBLK],
                            kT[hp : hp + 64, b * T + kt * 128 : b * T + (kt + 1) * 128],
                            qT[hp : hp + 64, t0 + col0 : t0 + TBLK],
                            start=True,
                            stop=True,
                        )
                    nc.scalar.activation(
                        es[:, :, col0:TBLK],
                        sT[:, :, col0:TBLK],
                        mybir.ActivationFunctionType.Exp,
                        scale=0.125,
                    )
                    if s >= 0:
                        # zero strictly-above-diagonal: keep es[p,h,col] iff col>=p
                        nc.gpsimd.affine_select(
                            out=es[:, :, col0 : col0 + 128],
                            in_=es[:, :, col0 : col0 + 128],
                            compare_op=mybir.AluOpType.is_ge,
                            fill=0.0,
                            base=0,
                            pattern=[[0, HPC], [1, 128]],
                            channel_multiplier=-1,
                        )
                    return es, col0

                def pv(kt, es, col0):
                    ktg = b * KT + kt
                    for h in range(HPC):
                        vsta = vN[:, ktg, 0:2, :] if h == 0 else vN[:, ktg, 2:4, :]
                        nc.tensor.matmul(
                            O_ps[:, h, col0:TBLK],
                            vsta,
                            es[:, h, col0:TBLK],
                            start=(kt == 0),
                            stop=(kt == nkt - 1),
                        )

                for kt in range(nkt):
                    cur = (kt, *scores_exp(kt))
                    if prev is not None:
                        pv(*prev)
                    prev = cur
                    emit_filler(2)
                pv(*prev)

                # normalize: O / rowsum (rowsum rows: h0 -> 0:64, h1 -> 64:128)
                rs = rinp.tile([128, TBLK], F32, tag="rs")
                rin = rinp.tile([128, TBLK], F32, tag="rin")
                nc.vector.tensor_copy(rs[0:64, :], O_ps[0:64, 0, :])
                nc.vector.tensor_copy(rs[64:128, :], O_ps[64:128, 1, :])
                nc.vector.reciprocal_approx_fast(rin[:], rs[:])
                nc.vector.tensor_mul(
                    oN[0:64, t0 : t0 + TBLK], O_ps[64:128, 0, :], rin[0:64, :]
                )
                nc.vector.tensor_mul(
                    oN[64:128, t0 : t0 + TBLK], O_ps[0:64, 1, :], rin[64:128, :]
                )
                queue_outproj(b, qb, spread=(b == 1 and qb == 0))

            # ---- the stream
            for tb in range(NTB):
                queue_qkv(tb)
            # prologue: get qkv(0) fully emitted before first attention
            drain_filler_until(0)
            for b, qb in (
                (0, 0), (0, 1), (0, 2), (0, 3),
                (1, 1), (1, 2), (1, 3), (1, 0),
            ):
                attn(b, qb)
            emit_filler(10**9)

    nc.compile()
    return nc


_NC_CACHE = None


def get_program():
    global _NC_CACHE
    if _NC_CACHE is None:
        _NC_CACHE = build_program()
    return _NC_CACHE


def make_in_maps(x, Wq, Wk, Wv, Wo):
    bf = ml_dtypes.bfloat16
    # xt layout [p, tb, ct, t] so each per-tb DMA is 8KB/partition contiguous
    xt = np.asarray(x, np.float32).reshape(NT, C).T  # [C, NT]
    xt = np.ascontiguousarray(
        xt.reshape(CT, 128, NTB, TBLK).transpose(1, 2, 0, 3)
    ).astype(bf)
    wq_b = np.asarray(Wq, np.float32).astype(bf)
    wk_b = np.asarray(Wk, np.float32).astype(bf)
    wv_b = np.asarray(Wv, np.float32).astype(bf)
    wo_b = np.asarray(Wo, np.float32).astype(bf)
    in_maps = []
    for cid in range(N_CORES):
        sl = slice(cid * FPC, (cid + 1) * FPC)
        in_maps.append(
            {
                "xt": xt,
                "wq": np.ascontiguousarray(
                    wq_b[:, sl].reshape(CT, 128, FPC).transpose(1, 0, 2)
                ),
                "wk": np.ascontiguousarray(
                    wk_b[:, sl].reshape(CT, 128, FPC).transpose(1, 0, 2)
                ),
                "wv": np.ascontiguousarray(
                    wv_b[:, sl].reshape(CT, 128, FPC).transpose(1, 0, 2)
                ),
                "wo": np.ascontiguousarray(wo_b[sl, :]),
            }
        )
    return in_maps


def kernel(x, Wq, Wk, Wv, Wo, bo, _trace=False, _tmpdir=None):
    x = np.asarray(x, dtype=np.float32)
    in_maps = make_in_maps(x, Wq, Wk, Wv, Wo)
    nc = get_program()
    res = run_bass_kernel_spmd(
        nc, in_maps, core_ids=list(range(N_CORES)), trace=_trace, tmpdir=_tmpdir
    )
    acc = res.results[0]["outT"].astype(np.float32)
    for i in range(1, N_CORES):
        acc = acc + res.results[i]["outT"].astype(np.float32)
    # acc [p, tb, ct, t] -> outT [C, NT] with c = ct*128+p, t = tb*512+ti
    outT = acc.transpose(2, 0, 1, 3).reshape(C, NT)
    out = outT.T + np.asarray(bo, np.float32)[None, :]
    if _trace:
        kernel._last_results = res
    return out.reshape(B, T, C).astype(np.float32)
